# revision 17
# baseline (speedup 1.0000x reference)
"""Trainium2 Bass kernel for CausalCrossAttention (B=8, T=769, C=1024, H=16).

Sharding: data-parallel over batch B=8 across the 8 NeuronCores (one batch
element per core, SPMD — identical program, different input slices).

Per-core dataflow (all matmuls in fp32r on the PE at 1 cycle/row):
  1. Q/K projections in [c_out, t] layout:  psum[co,t] = sum_ci WT[ci,co]^T? ...
     matmul(out=[co_tile, t], lhsT=W^T[ci, co_tile], rhs=x^T[ci, t]).
     Host pre-transposes W and x so the contraction dim (ci) is the SBUF
     partition dim for both operands (avoids all on-chip transposes).
     Rotary is applied in [dim, t] layout via a host permutation of W's output
     dims (even/odd pair split) + partition-block-swap DMAs + 3 DVE ops.
  2. V projection in [t, c_out] layout (lhsT=x^T tile, rhs=W^T), written into a
     ones-augmented V buffer [t, 16, 65] (col 64 = 1.0 -> softmax denominator
     comes out of the PV matmul for free).
  3. Attention per head in S^T layout: S^T[tkv, tq] = k^T.T @ q^T
     (K=64 contraction). exp on ACT (scale=1/8 folded), prefix-causal mask
     (col < 256 + row) applied as a triangular multiply on boundary tiles,
     O_aug^T[65, tq] = V_aug.T @ P^T accumulated over kv tiles; row 64 is the
     softmax denominator. Division via DVE with a DMA-broadcast reciprocal.
  4. Output projection from the Y^T[ci, t] layout that PV naturally produces.
"""

import os

import numpy as np

B, T, C = 8, 769, 1024
H, HD, L = 16, 64, 32
COND = 256
NCI = 8  # 1024 / 128 contraction tiles
NCO = 8
NTT = 7  # t tiles: 6 full + 1 single row
TP = 770  # streamed T padded to even (fp32r matmul: moving N must be even)
R0 = (0, 512)
R1 = (512, 770)

_CACHE = {}


def _build_program():
    import concourse.mybir as mybir
    import concourse.tile as tile
    from concourse import bacc

    f32 = mybir.dt.float32
    f32r = mybir.dt.float32r
    Exp = mybir.ActivationFunctionType.Exp
    Ident = mybir.ActivationFunctionType.Identity

    def r(ap):
        # tiles feeding matmuls are declared float32r natively; walrus
        # requires producers (DMA/ACT/DVE) to emit rounded fp32r data.
        return ap

    nc = bacc.Bacc("TRN2", target_bir_lowering=False)

    xqT_d = nc.dram_tensor("xqT", [C, TP], f32r, kind="ExternalInput")
    xkvT_d = nc.dram_tensor("xkvT", [C, TP], f32r, kind="ExternalInput")
    wq_d = nc.dram_tensor("wqT", [C, C], f32r, kind="ExternalInput")
    wk_d = nc.dram_tensor("wkT", [C, C], f32r, kind="ExternalInput")
    wv_d = nc.dram_tensor("wvT", [C, C], f32r, kind="ExternalInput")
    wp_d = nc.dram_tensor("wpT", [C, C], f32r, kind="ExternalInput")
    bq_d = nc.dram_tensor("bq2", [128, NCO], f32, kind="ExternalInput")
    bk_d = nc.dram_tensor("bk2", [128, NCO], f32, kind="ExternalInput")
    bv_d = nc.dram_tensor("bv1", [1, C], f32, kind="ExternalInput")
    bp_d = nc.dram_tensor("bp1", [1, C], f32, kind="ExternalInput")
    cos_d = nc.dram_tensor("cosP", [128, TP], f32, kind="ExternalInput")
    sin_d = nc.dram_tensor("sinP", [128, TP], f32, kind="ExternalInput")
    m0_d = nc.dram_tensor("m0", [128, 128], f32, kind="ExternalInput")
    out_d = nc.dram_tensor("out", [T, C], f32, kind="ExternalOutput")

    # Per-(kv-tile) q ranges in the 0:512 block + mask offset (see module doc).
    # nk covers kv cols [128*nk, 128*nk+128); allowed iff kv_col < 256 + q_col,
    # i.e. p < f + 256 - 128*nk with p the in-tile kv index, f the abs q col.
    R0SUB = {0: (0, 512, None), 1: (0, 512, None), 2: (0, 512, 0),
             3: (128, 512, 128), 4: (256, 512, 256), 5: (384, 512, 384)}

    with tile.TileContext(nc) as tc:
        with (
            tc.tile_pool(name="consts", bufs=1) as consts,
            tc.tile_pool(name="wpool", bufs=1) as wpool,
            tc.tile_pool(name="qkpool", bufs=1) as qkpool,
            tc.tile_pool(name="vpool", bufs=1) as vpool,
            tc.tile_pool(name="ypool", bufs=1) as ypool,
        ):
            cos_sb = consts.tile([128, TP], f32, tag="cos")
            sin_sb = consts.tile([128, TP], f32, tag="sin")
            m0_sb = consts.tile([128, 128], f32, tag="m0")
            bq_sb = consts.tile([128, NCO], f32, tag="bq")
            bk_sb = consts.tile([128, NCO], f32, tag="bk")
            ones16 = consts.tile([128, 16], f32, tag="ones16")
            nc.vector.memset(ones16, 1.0)
            zero2 = consts.tile([1, 2], f32, tag="zero2")
            nc.vector.memset(zero2, 0.0)
            nc.sync.dma_start(out=cos_sb, in_=cos_d[:, :])
            nc.sync.dma_start(out=sin_sb, in_=sin_d[:, :])
            nc.sync.dma_start(out=m0_sb, in_=m0_d[:, :])
            nc.sync.dma_start(out=bq_sb, in_=bq_d[:, :])
            nc.sync.dma_start(out=bk_sb, in_=bk_d[:, :])

            qT = qkpool.tile([128, NCI, TP], f32r, tag="qT")
            kT = qkpool.tile([128, NCI, TP], f32r, tag="kT")
            vaug = vpool.tile([128, NTT, H, HD + 1], f32r, tag="vaug")
            yT = ypool.tile([128, NCI, TP], f32r, tag="yT")

            def load_w(wdram):
                ws = []
                for ci in range(NCI):
                    wt = wpool.tile([128, C], f32r, tag=f"w{ci}")
                    nc.sync.dma_start(
                        out=wt, in_=wdram[ci * 128:(ci + 1) * 128, :])
                    ws.append(wt)
                return ws

            def proj_qk(w, x, b_sb, outT, psA):
                """outT[:, co, :] = W @ x^T + b, then partial rotary."""
                for co in range(NCO):
                    ps = psA.tile([128, 1024], f32, tag="ps")
                    for ci in range(NCI):
                        lhs = r(w[ci][:, co * 128:(co + 1) * 128])
                        for (lo, hi) in (R0, R1):
                            nc.tensor.matmul(
                                ps[:, lo:hi], lhs, r(x[:, ci, lo:hi]),
                                start=(ci == 0), stop=(ci == NCI - 1))
                    # bias add + PSUM->SBUF on ACT
                    nc.scalar.activation(
                        out=outT[:, co, :], in_=ps[:, 0:TP], func=Ident,
                        bias=b_sb[:, co:co + 1], scale=1.0)
                    # rotary: swap 16-row blocks of the (host-permuted) rotary
                    # dims, then q = q*cos + swapped*sin.  cos rows outside the
                    # rotary dims are 1.0, sin rows are 0.0, so full-height DVE
                    # ops are safe (and cost the same as 32-row ones).
                    sh = shpool.tile([128, TP], f32r, tag="sh")
                    # rows 32:64 are pass-through dims; sin there is 0 but the
                    # row must be initialized for the full-height DVE ops.
                    nc.sync.dma_start(
                        out=sh[32:64, :], in_=outT[32:64, co, :])
                    for s in (0, 64):
                        nc.sync.dma_start(
                            out=sh[s:s + 16, :], in_=outT[s + 16:s + 32, co, :])
                        nc.sync.dma_start(
                            out=sh[s + 16:s + 32, :], in_=outT[s:s + 16, co, :])
                    nc.vector.tensor_mul(sh[0:96, :], sh[0:96, :], sin_sb[0:96, :])
                    nc.vector.tensor_mul(
                        outT[:, co, :], outT[:, co, :], cos_sb)
                    nc.vector.tensor_add(
                        outT[0:96, co, :], outT[0:96, co, :], sh[0:96, :])

            def proj_v(w, x, psA, bv_sb):
                for g in (range(0, 4), range(4, 7)):
                    pss = {}
                    for tt in g:
                        pss[tt] = psA.tile([128, 1024], f32, tag="ps", name=f"psv{tt}")
                    for ci in range(NCI):
                        for tt in g:
                            tsz = 128 if tt < 6 else 1
                            lhs = r(x[:, ci, tt * 128:tt * 128 + tsz])
                            for lo in (0, 512):
                                nc.tensor.matmul(
                                    pss[tt][:tsz, lo:lo + 512], lhs,
                                    r(w[ci][:, lo:lo + 512]),
                                    start=(ci == 0), stop=(ci == NCI - 1))
                    for tt in g:
                        tsz = 128 if tt < 6 else 1
                        nc.vector.tensor_add(
                            vaug[:tsz, tt, :, 0:HD],
                            pss[tt][:tsz, :].rearrange("p (h d) -> p h d", h=H),
                            bv_sb[:tsz, :].rearrange("p (h d) -> p h d", h=H))
                        nc.vector.tensor_copy(
                            vaug[:tsz, tt, :, HD:HD + 1],
                            ones16[:tsz, :].unsqueeze(2))

            def attn(h, pt_pool, psS, psO, dnd, rd_pool):
                j, s = h // 2, 64 * (h % 2)
                kh = kT[s:s + 64, j, :]
                qh = qT[s:s + 64, j, :]
                pts = {}
                for nk in range(6):
                    pt = pt_pool.tile([128, TP], f32r, tag=f"pt{nk}", name=f"pt{nk}")
                    pts[nk] = pt
                    qlo, qhi, moff = R0SUB[nk]
                    ps = psS.tile([128, 512], f32, tag="st")
                    nc.tensor.matmul(
                        ps[:, 0:qhi - qlo], r(kh[:, nk * 128:(nk + 1) * 128]),
                        r(qh[:, qlo:qhi]), start=True, stop=True)
                    nc.scalar.activation(
                        out=pt[:, qlo:qhi], in_=ps[:, 0:qhi - qlo],
                        func=Exp, scale=0.125)
                    if moff is not None:
                        nc.vector.tensor_mul(
                            pt[:, moff:moff + 128], pt[:, moff:moff + 128],
                            m0_sb)
                    ps2 = psS.tile([128, 512], f32, tag="st")
                    nc.tensor.matmul(
                        ps2[:, 0:258], r(kh[:, nk * 128:(nk + 1) * 128]),
                        r(qh[:, 512:770]), start=True, stop=True)
                    nc.scalar.activation(
                        out=pt[:, 512:770], in_=ps2[:, 0:258],
                        func=Exp, scale=0.125)
                # kv col 768 (single kv row): q col 512 is masked; stream the
                # full even range and zero that one probability instead.
                pt6 = pt_pool.tile([128, TP], f32r, tag="pt6", bufs=1)
                ps = psS.tile([128, 512], f32, tag="st")
                nc.tensor.matmul(
                    ps[0:1, 0:258], r(kh[:, 768:769]), r(qh[:, 512:770]),
                    start=True, stop=True)
                nc.scalar.activation(
                    out=pt6[0:1, 513:770], in_=ps[0:1, 1:258],
                    func=Exp, scale=0.125)
                nc.vector.tensor_copy(pt6[0:1, 512:513], zero2[0:1, 0:1])

                # PV (+denominator via the ones column), per q range
                o0 = psO.tile([HD + 1, 512], f32, tag="ov")
                for nk in range(6):
                    qlo, qhi, _ = R0SUB[nk]
                    nc.tensor.matmul(
                        o0[:, qlo:qhi], r(vaug[:, nk, h, :]),
                        r(pts[nk][:, qlo:qhi]),
                        start=(nk == 0), stop=(nk == 5))
                o1 = psO.tile([HD + 1, 512], f32, tag="ov")
                for nk in range(6):
                    nc.tensor.matmul(
                        o1[:, 0:258], r(vaug[:, nk, h, :]),
                        r(pts[nk][:, 512:770]),
                        start=(nk == 0), stop=False)
                nc.tensor.matmul(
                    o1[:, 0:258], r(vaug[0:1, 6, h, :]),
                    r(pt6[0:1, 512:770]), start=False, stop=True)

                for (lo, hi), o in ((R0, o0), (R1, o1)):
                    n = hi - lo
                    # unnormalized numerator into yT; denominator row to DRAM
                    # (batched reciprocal + broadcast happens after all heads)
                    nc.scalar.activation(
                        out=yT[s:s + 64, j, lo:hi], in_=o[0:HD, 0:n],
                        func=mybir.ActivationFunctionType.Copy)
                    stg = rd_pool.tile([1, TP], f32, tag="dstage", bufs=4,
                                       name=f"stg{h}")
                    nc.scalar.activation(
                        out=stg[0:1, lo:hi], in_=o[HD:HD + 1, 0:n],
                        func=mybir.ActivationFunctionType.Copy)
                    nc.sync.dma_start(
                        out=dnd[h:h + 1, lo:hi], in_=stg[0:1, lo:hi])

            def proj_out(w, psA, opool, bp_sb):
                for g in (range(0, 4), range(4, 7)):
                    pss = {}
                    for tt in g:
                        pss[tt] = psA.tile([128, 1024], f32, tag="pso", name=f"pso{tt}")
                    for ci in range(NCI):
                        for tt in g:
                            tsz = 128 if tt < 6 else 1
                            lhs = r(yT[:, ci, tt * 128:tt * 128 + tsz])
                            for lo in (0, 512):
                                nc.tensor.matmul(
                                    pss[tt][:tsz, lo:lo + 512], lhs,
                                    r(w[ci][:, lo:lo + 512]),
                                    start=(ci == 0), stop=(ci == NCI - 1))
                    for tt in g:
                        tsz = 128 if tt < 6 else 1
                        ot = opool.tile([128, 1024], f32, tag="ot")
                        nc.vector.tensor_add(
                            ot[:tsz, :], pss[tt][:tsz, :], bp_sb[:tsz, :])
                        nc.sync.dma_start(
                            out=out_d[tt * 128:tt * 128 + tsz, :],
                            in_=ot[:tsz, :])

            # ---- phase 1: projections ----
            with (
                tc.tile_pool(name="psA", bufs=4, space="PSUM") as psA,
                tc.tile_pool(name="xq", bufs=1) as xqp,
                tc.tile_pool(name="xkv", bufs=1) as xkp,
                tc.tile_pool(name="shpool", bufs=2) as shpool,
            ):
                bv_sb = xqp.tile([128, C], f32, tag="bv")
                nc.gpsimd.dma_start(
                    out=bv_sb, in_=bv_d[0:1, :].broadcast_to((128, C)))
                xq = xqp.tile([128, NCI, TP], f32r, tag="xq")
                xkv = xkp.tile([128, NCI, TP], f32r, tag="xkv")
                # per-ci-tile DMAs so the first matmuls start early
                for ci in range(NCI):
                    nc.sync.dma_start(
                        out=xq[:, ci, :],
                        in_=xqT_d[ci * 128:(ci + 1) * 128, :])
                    nc.sync.dma_start(
                        out=xkv[:, ci, :],
                        in_=xkvT_d[ci * 128:(ci + 1) * 128, :])
                wq = load_w(wq_d)
                proj_qk(wq, xq, bq_sb, qT, psA)
                wk = load_w(wk_d)
                proj_qk(wk, xkv, bk_sb, kT, psA)
                wv = load_w(wv_d)
                proj_v(wv, xkv, psA, bv_sb)

            # ---- phase 2: attention ----
            with (
                tc.tile_pool(name="ptp", bufs=2) as pt_pool,
                tc.tile_pool(name="psS", bufs=4, space="PSUM") as psS,
                tc.tile_pool(name="psO", bufs=4, space="PSUM") as psO,
                tc.tile_pool(name="rdp", bufs=1) as rd_pool,
                tc.tile_pool(name="rdbcp", bufs=1) as rdbc_pool,
                tc.tile_pool(name="rddp", bufs=1, space="DRAM") as dram_pool,
            ):
                dnd = dram_pool.tile([H, TP], f32, tag="dnd")
                for h in range(H):
                    attn(h, pt_pool, psS, psO, dnd, rd_pool)
                # batched softmax denominator: one reciprocal for all heads
                dn_sb = rd_pool.tile([H, TP], f32, tag="dn")
                nc.sync.dma_start(out=dn_sb, in_=dnd[:, :])
                rda = rd_pool.tile([H, TP], f32, tag="rda")
                nc.vector.reciprocal(rda, dn_sb)
                rdad = dram_pool.tile([H, TP], f32, tag="rdad")
                nc.sync.dma_start(out=rdad[:, :], in_=rda)
                for j in range(NCI):
                    rdbc = rdbc_pool.tile([128, TP], f32, tag="rdbc",
                                          name=f"rdbc{j}")
                    nc.gpsimd.dma_start(
                        out=rdbc[0:64, :],
                        in_=rdad[2 * j:2 * j + 1, :].broadcast_to((64, TP)))
                    nc.gpsimd.dma_start(
                        out=rdbc[64:128, :],
                        in_=rdad[2 * j + 1:2 * j + 2, :].broadcast_to((64, TP)))
                    for s in (0, 64):
                        nc.vector.tensor_mul(
                            yT[s:s + 64, j, :], yT[s:s + 64, j, :],
                            rdbc[s:s + 64, :])

            # ---- phase 3: output projection ----
            with (
                tc.tile_pool(name="psB", bufs=4, space="PSUM") as psB,
                tc.tile_pool(name="opool", bufs=3) as opool,
            ):
                bp_sb = opool.tile([128, C], f32, tag="bp")
                nc.gpsimd.dma_start(
                    out=bp_sb, in_=bp_d[0:1, :].broadcast_to((128, C)))
                wp = load_w(wp_d)
                proj_out(wp, psB, opool, bp_sb)

    nc.compile()
    return nc


def _host_prep(x_q, x_kv, rotary_pos_emb, Wq, bq, Wk, bk, Wv, bv, Wp, bp):
    f = np.float32
    x_q = np.asarray(x_q, f)
    x_kv = np.asarray(x_kv, f)
    freqs = np.asarray(rotary_pos_emb, f)

    # Even/odd pair-split permutation of the first 32 dims of each head, so
    # rotate_half becomes a 16-partition block swap on chip.
    perm = np.arange(C)
    for h in range(H):
        b0 = h * HD
        blk = np.empty(HD, np.int64)
        blk[0:16] = b0 + np.arange(0, 32, 2)
        blk[16:32] = b0 + np.arange(1, 32, 2)
        blk[32:64] = b0 + np.arange(32, 64)
        perm[b0:b0 + HD] = blk

    def wT(W, p=None):
        W = np.asarray(W, f)
        if p is not None:
            W = W[p, :]
        return np.ascontiguousarray(W.T)

    cosE = np.cos(freqs[:, 0::2]).T  # [16, T]
    cosO = np.cos(freqs[:, 1::2]).T
    sinE = -np.sin(freqs[:, 0::2]).T
    sinO = np.sin(freqs[:, 1::2]).T
    cosP = np.ones((128, TP), f)
    sinP = np.zeros((128, TP), f)
    for s in (0, 64):
        cosP[s:s + 16, :T] = cosE
        cosP[s + 16:s + 32, :T] = cosO
        sinP[s:s + 16, :T] = sinE
        sinP[s + 16:s + 32, :T] = sinO

    p_idx = np.arange(128)[:, None]
    f_idx = np.arange(128)[None, :]
    m0 = (p_idx < f_idx).astype(f)

    bqp = np.asarray(bq, f)[perm]
    bkp = np.asarray(bk, f)[perm]
    shared = {
        "wqT": wT(Wq, perm),
        "wkT": wT(Wk, perm),
        "wvT": wT(Wv),
        "wpT": wT(Wp),
        "bq2": np.ascontiguousarray(bqp.reshape(NCO, 128).T),
        "bk2": np.ascontiguousarray(bkp.reshape(NCO, 128).T),
        "bv1": np.asarray(bv, f).reshape(1, C).copy(),
        "bp1": np.asarray(bp, f).reshape(1, C).copy(),
        "cosP": np.ascontiguousarray(cosP),
        "sinP": np.ascontiguousarray(sinP),
        "m0": np.ascontiguousarray(m0),
    }
    def padT(xt):
        out = np.zeros((C, TP), f)
        out[:, :T] = xt
        return out

    in_maps = []
    for b in range(B):
        m = dict(shared)
        m["xqT"] = padT(x_q[b].T)
        m["xkvT"] = padT(x_kv[b].T)
        in_maps.append(m)
    return in_maps


def kernel(x_q, x_kv, rotary_pos_emb, Wq, bq, Wk, bk, Wv, bv, Wp, bp):
    from concourse.bass_utils import run_bass_kernel_spmd

    if "nc" not in _CACHE:
        _CACHE["nc"] = _build_program()
    nc = _CACHE["nc"]

    in_maps = _host_prep(x_q, x_kv, rotary_pos_emb,
                         Wq, bq, Wk, bk, Wv, bv, Wp, bp)
    trace = os.environ.get("BTK_TRACE", "0") == "1"
    res = run_bass_kernel_spmd(
        nc, in_maps, core_ids=list(range(B)), trace=trace)
    _CACHE["last_result"] = res
    return np.stack([r["out"] for r in res.results], axis=0)
